# revision 5
# baseline (speedup 1.0000x reference)
"""Trainium2 Bass kernel for the masked per-site stencil contraction

    y[o, n] = f( sum_{i,k} Wconv[o,i,k] * mask[n,o,i,k] * x[i, shifts[n,k]] + bconv[o] )
    f(v) = (sigmoid(v) - 0.5) * (2 + 2e)/(e - 1) = (2+2e)/(2(e-1)) * tanh(v/2)

Shapes: O=I=32, K=13, N=4096.  Sharded over 8 NeuronCores along the site
dimension N (NS=512 sites per core); mask/shifts/output partitioned along N,
x/Wconv/bconv replicated (folded into the per-core gather/weight uploads).

Per-core design:
  * The gather g[i,k,n] = x[i, shifts[n,k]] is pure data movement -> done
    host-side during input layout (no GPSIMD ap_gather); uploaded fp16,
    de-replicated [128, 4, NS] and expanded to 13 mask-mirroring t-slices
    on-device with DVE copies (4x mode, ~free).
  * mask shipped as fp8_e4m3 (0/1 values exact) and cast to fp16 inside the
    SWDGE (gpsimd) DMA: halves HBM+wire bytes with zero extra compute.
    Layout: one contiguous [128, 13*NS] block per 4-channel group og;
    partition p = (kk,i) with kk=p//32, i=p%32; t-slice t=4c+j holds
    (k=4c+kk, channel 4og+j) for c<3, t=12 holds k=12 with the channel
    packed as j=p//32.  All 8 group DMAs pre-issued (no FIFO head-of-line
    blocking); staging/output DMAs ride the sync (HWDGE) ring.
  * DVE: two [128, *, NS] fp16 multiplies per group (2x mode): prod = mask*g.
  * PE:  col-tiled — the 4 channels of a group run CONCURRENTLY in distinct
    32-column groups of the array (tile_position=(0, 32j)), each a 4-matmul
    f32 accumulation chain at PSUM partition 32j of one shared bank
    (lhsT = fp16 weight column [128, 1], rhs = prod t-slice [128, NS]).
  * ACT: per-channel tanh(0.5*v + 0.5*b) from PSUM partition 32j into an
    SBUF tile at the same partition (engines are partition-aligned; bias
    pre-placed at partitions 32j host-side), then one partition-strided
    staging DMA per group into ystage; final DVE scale by (1+e)/(e-1);
    DMA out.
"""

import math

import numpy as np

import concourse.bacc as bacc
import concourse.mybir as mybir
from concourse import tile
from concourse.bass_utils import run_bass_kernel_spmd

O, I, K, N = 32, 32, 13, 4096
NCORES = 8
NS = N // NCORES
NG = O // 4
_E = math.e
SCALE = (2.0 + 2.0 * _E) / (_E - 1.0)

_F32 = mybir.dt.float32
_F16 = mybir.dt.float16
_F8 = mybir.dt.float8e4

_BUILT = {}
TSPLIT = 8


def _declare(nc):
    d = {}
    d["maskp"] = nc.declare_dram_parameter(
        "maskp", [NG, 128, K, NS], _F8, isOutput=False
    )
    d["gb4"] = nc.declare_dram_parameter("gb4", [128, 4, NS], _F16, isOutput=False)
    d["wf"] = nc.declare_dram_parameter("wf", [128, NG, 4, 4], _F16, isOutput=False)
    d["brow4"] = nc.declare_dram_parameter("brow4", [128, NG], _F32, isOutput=False)
    d["y"] = nc.declare_dram_parameter("y", [O, NS], _F32, isOutput=True)
    return d


def _emit(nc, tc, d, pools, dma_only=False):
    cpool, mpool, ppool, opool, qpool = pools

    gb4 = cpool.tile([128, 4, NS], _F16, tag="gb4")
    nc.scalar.dma_start(gb4[:, :, :], d["gb4"][:, :, :])
    wf = cpool.tile([128, NG, 4, 4], _F16, tag="wf")
    nc.sync.dma_start(wf[:, :, :, :], d["wf"][:, :, :, :])
    brow4 = cpool.tile([128, NG], _F32, tag="brow4")
    nc.sync.dma_start(brow4[:, :], d["brow4"][:, :])
    bh2x = cpool.tile([128, NG], _F32, tag="bh2x")
    nc.scalar.activation(
        bh2x[:, :], brow4[:, :], mybir.ActivationFunctionType.Copy, scale=0.5
    )

    # expand g to the 13 mask-mirroring t-slices (DVE copy, 4x mode)
    gbf = cpool.tile([128, K, NS], _F16, tag="gbfx")
    for t in range(K):
        c = t // 4 if t < 12 else 3
        nc.vector.tensor_copy(gbf[:, t, :], gb4[:, c, :])

    ystage = opool.tile([O, NS], _F32, tag="ys")

    # pre-issue all mask DMAs on the SWDGE ring (fp8 -> fp16 cast in-DMA)
    mgs = []
    for og in range(NG):
        mg = mpool.tile([128, K, NS], _F16, tag="mg", bufs=NG)
        nc.gpsimd.dma_start(mg[:, :, :], d["maskp"][og, :, :, :])
        mgs.append(mg)

    if dma_only:
        dummy = opool.tile([1, NS], _F32, tag="dummy")
        nc.vector.tensor_copy(dummy[:, :], mgs[-1][0:1, 0, :])
        nc.sync.dma_start(d["y"][0:1, :], dummy[:, :])
        return

    for og in range(NG):
        mg = mgs[og]
        pr = ppool.tile([128, K, NS], _F16, tag="pr", bufs=2)
        nc.vector.tensor_mul(pr[:, :TSPLIT, :], mg[:, :TSPLIT, :], gbf[:, :TSPLIT, :])
        nc.vector.tensor_mul(pr[:, TSPLIT:, :], mg[:, TSPLIT:, :], gbf[:, TSPLIT:, :])
        # col-tiled PE: the 4 channels run concurrently in distinct 32-col
        # groups of the array, chains at PSUM partitions 32j of ONE bank
        ypt = qpool.tile([128, NS], _F32, tag="ypt", bufs=4)
        for c in range(4):
            for j in range(4):
                rhs = pr[:, 4 * c + j, :] if c < 3 else pr[:, 12, :]
                nc.tensor.matmul(
                    ypt[32 * j : 32 * j + 1, :], wf[:, og, c, j : j + 1], rhs,
                    start=(c == 0), stop=(c == 3), tile_position=(0, 32 * j),
                )
        ycat4 = opool.tile([128, NS], _F32, tag="ycat4", bufs=2)
        for j in range(4):
            nc.scalar.activation(
                ycat4[32 * j : 32 * j + 1, :], ypt[32 * j : 32 * j + 1, :],
                mybir.ActivationFunctionType.Tanh,
                bias=bh2x[32 * j : 32 * j + 1, og : og + 1], scale=0.5,
            )
        nc.sync.dma_start(
            ystage[4 * og : 4 * og + 4, :],
            ycat4.rearrange("(a b) n -> a b n", b=32)[:, 0, :],
        )

    nc.vector.tensor_scalar_mul(ystage[:, :], ystage[:, :], SCALE / 2.0)
    nc.sync.dma_start(d["y"][:, :], ystage[:, :])


def _pools(tc, stack):
    names = [("const", 1), ("mask", 1), ("prod", 1), ("out", 1), ("psum", 1)]
    pools = []
    for name, bufs in names:
        kw = {"space": "PSUM"} if name == "psum" else {}
        pools.append(stack.enter_context(tc.tile_pool(name=name, bufs=bufs, **kw)))
    return pools


def _build(reps=1, dma_only=False):
    key = ("nc", reps, dma_only)
    if key in _BUILT:
        return _BUILT[key]
    from contextlib import ExitStack

    nc = bacc.Bacc("TRN2", target_bir_lowering=False, debug=False)
    d = _declare(nc)
    with tile.TileContext(nc) as tc:
        with ExitStack() as stack:
            pools = _pools(tc, stack)
            for _ in range(reps):
                _emit(nc, tc, d, pools, dma_only=dma_only)
    nc.compile()
    _BUILT[key] = nc
    return nc


def make_in_maps(x, Wconv, bconv, mask, shifts):
    """Host-side shard/layout prep: pure data movement + dtype casts
    (mask to fp8_e4m3 and weights/gathered-x to fp16 -- 0/1 mask values
    are exact in fp8)."""
    import ml_dtypes

    x = np.asarray(x, dtype=np.float32)
    W = np.asarray(Wconv, dtype=np.float32)
    mask = np.asarray(mask, dtype=np.float32)
    shifts = np.asarray(shifts)

    ii = np.arange(128) % 32
    kk = np.arange(128) // 32

    wf = np.zeros((128, NG, 4, 4), np.float16)
    for og in range(NG):
        for j in range(4):
            o = 4 * og + j
            for c in range(3):
                wf[:, og, c, j] = W[o, ii, 4 * c + kk]
            wf[:, og, 3, j] = np.where(kk == j, W[o, ii, 12], 0.0)

    brow4 = np.zeros((128, NG), np.float32)
    for og in range(NG):
        for j in range(4):
            brow4[32 * j, og] = float(bconv[4 * og + j])

    in_maps = []
    for core in range(NCORES):
        sl = slice(core * NS, (core + 1) * NS)
        m = mask[sl]
        sh = shifts[sl]

        maskp = np.empty((NG, 128, K, NS), ml_dtypes.float8_e4m3)
        mb = m[:, :, :, :12].reshape(NS, NG, 4, I, 3, 4)
        maskp[:, :, :12, :] = (
            mb.transpose(1, 5, 3, 4, 2, 0).reshape(NG, 128, 12, NS)
        )
        m12 = m[:, :, :, 12].reshape(NS, NG, 4, I)
        maskp[:, :, 12, :] = m12.transpose(1, 2, 3, 0).reshape(NG, 128, NS)
        maskp = np.ascontiguousarray(maskp)

        gb4 = np.empty((128, 4, NS), np.float16)
        for c in range(3):
            gb4[:, c, :] = x[ii[:, None], sh[:, 4 * c + kk].T]
        gb4[:, 3, :] = x[ii[:, None], np.broadcast_to(sh[:, 12], (128, NS))]
        gb4 = np.ascontiguousarray(gb4)

        in_maps.append({"maskp": maskp, "gb4": gb4, "wf": wf, "brow4": brow4})
    return in_maps


def kernel(x, Wconv, bconv, mask, shifts):
    nc = _build()
    in_maps = make_in_maps(x, Wconv, bconv, mask, shifts)
    res = run_bass_kernel_spmd(nc, in_maps, core_ids=list(range(NCORES)))
    y = np.empty((O, N), np.float32)
    for core in range(NCORES):
        y[:, core * NS : (core + 1) * NS] = res.results[core]["y"]
    return y
